# revision 19
# baseline (speedup 1.0000x reference)
"""Trainium2 Bass kernel for a dense transformer block (nn_Block_29583734734992).

Reference computation (fp32):
    resid = resid + Attn(LN1(resid))          # 16 heads, d_head 64, causal
    resid = resid + MLP(LN2(resid)) + b_out   # d_mlp 4096, tanh-gelu

Sharding over 8 NeuronCores (v2 — token-sharded LN/QKV + head-sharded attention):
  - Phase A (token-parallel): core c owns 512 tokens (rows [256c, 256c+256) of
    each batch). It LN1s + transposes ONLY those tokens and computes the QKV
    projections for ALL 16 heads on them (same total FLOPs as head-parallel
    QKV, but the LN/transpose work is sharded 8x instead of replicated 8x).
    V is pre-transposed to token-major on the sender. One AllToAll per batch
    reshards q/k/v to head-owners.
  - Phase B (head-parallel): core c holds heads (2c, 2c+1) with full-sequence
    qT/kT (feature-major) and token-major V. Causal scores/softmax/z as in
    v1, but the two heads' score tiles share one 2-bank PSUM tile so a single
    ACT exp covers both ([128,1024] per key-chunk). Two AllToAlls (one per
    batch) reshard z back to token-owners; each fires as soon as that batch's
    attention is staged so it hides under the other batch / post compute.
  - Phase C (token-parallel): o-projection, residual add, LN2 and the full
    MLP for the core's 512 tokens, writing a [512, 1024] output shard.

Numerics: bf16 matmuls with fp32 PSUM accumulation throughout; LN scale/bias,
the 1/sqrt(64) softmax scale and b_in are folded into weights / activation
biases on the host. Softmax skips max-subtraction (scores are small) and
applies the causal mask multiplicatively after exp; the per-query softmax
denominator comes from an extra ones-column appended to V's stationary
operand. DMAs are batched into multi-dim access patterns (each dma_start
costs ~0.7us of Sync-queue issue time).
"""

import sys

for _p in ("/opt/trn_rl_repo", "/root/.axon_site/_ro/trn_rl_repo"):
    if _p not in sys.path:
        sys.path.insert(0, _p)

import ml_dtypes
import numpy as np

import concourse.bass as bass
import concourse.mybir as mybir
import concourse.tile as tile
from concourse import bacc
from concourse.bass_utils import run_bass_kernel_spmd

F32 = mybir.dt.float32
F32R = mybir.dt.float32r
BF16 = mybir.dt.bfloat16
FP8 = mybir.dt.float8e4
WS_MLP = 64.0  # host scale on w_in/w_out so fp8e4m3 sees a good exponent range
AF = mybir.ActivationFunctionType
OP = mybir.AluOpType

N_CORES = 8
B, S, D = 2, 2048, 1024
H, DH, DM = 16, 64, 4096
EPS = 1e-5
HPC = H // N_CORES  # heads per core = 2
TSH = (B * S) // N_CORES  # tokens per core = 512 (256 from each batch)
TPB = TSH // B  # tokens per core per batch = 256
ND = D // 128  # 8 d_model chunks
NM = DM // 128  # 32 d_mlp chunks
NQC = S // 512  # 4 query chunks of 512
NKC = S // 128  # 16 key chunks of 128
NOC = 3 * N_CORES  # 24 QKV output chunks of 128 (dst-major: q,k,v per dst)

# Replace the A2A collectives with local DRAM copies so the module has no
# collectives (lets TimelineSim model a single core). Timing-analysis only.
FAKE_A2A = False


def build_nc(reps: int = 1):
    nc = bacc.Bacc(
        "TRN2",
        target_bir_lowering=False,
        debug=False,
        num_devices=1 if FAKE_A2A else N_CORES,
    )

    resid_mine = nc.dram_tensor("resid_mine", [TSH, D], F32, kind="ExternalInput")
    wqkv = nc.dram_tensor("wqkv", [ND, 128, 3 * D], BF16, kind="ExternalInput")
    bqkv = nc.dram_tensor("bqkv", [NOC, 128, 1], F32, kind="ExternalInput")
    wo = nc.dram_tensor("wo", [ND, 128, D], BF16, kind="ExternalInput")
    win = nc.dram_tensor("win", [NM // 4, ND, 128, 512], FP8, kind="ExternalInput")
    bin_ = nc.dram_tensor("bin", [128, NM], F32, kind="ExternalInput")
    wout = nc.dram_tensor("wout", [NM, 128, D], FP8, kind="ExternalInput")
    bout = nc.dram_tensor("bout", [1, D], F32R, kind="ExternalInput")
    masks = nc.dram_tensor("masks", [128, 256], BF16, kind="ExternalInput")
    ident = nc.dram_tensor("ident", [128, 128], F32, kind="ExternalInput")
    ones = nc.dram_tensor("ones", [1, 128], F32R, kind="ExternalInput")
    y = nc.dram_tensor("y", [TSH, D], F32, kind="ExternalOutput")

    with tile.TileContext(nc) as tc:
        with (
            tc.tile_pool(name="singles", bufs=1) as singles,
            tc.tile_pool(name="dram", bufs=1, space="DRAM") as dram,
        ):
            # A2A buffers. qkv block per dst j: rows 0:128 q feats (heads
            # 2j,2j+1), 128:256 k feats, 256:384 v in token-major packing
            # (row r, col tb*128+f  ->  v[token tb*128+r, feat f]).
            qkv_in = [
                dram.tile([N_CORES, 3 * 128, TPB], BF16, tag=f"qi{b}", name=f"qi{b}")
                for b in range(B)
            ]
            qkv_out = [
                dram.tile([N_CORES, 3 * 128, TPB], BF16, tag=f"qo{b}", name=f"qo{b}")
                for b in range(B)
            ]
            z_in = [
                dram.tile([N_CORES, HPC * DH, TPB], BF16, tag=f"zi{b}", name=f"zi{b}")
                for b in range(B)
            ]
            z_out = [
                dram.tile([N_CORES, HPC * DH, TPB], BF16, tag=f"zo{b}", name=f"zo{b}")
                for b in range(B)
            ]

            ident_sb = singles.tile([128, 128], F32)
            nc.sync.dma_start(ident_sb[:], ident[:])
            identb = singles.tile([128, 128], BF16)
            nc.vector.tensor_copy(identb[:], ident_sb[:])
            mask_sb = singles.tile([128, 256], BF16)
            nc.sync.dma_start(mask_sb[:], masks[:])
            wqkv_sb = singles.tile([128, ND, 3 * D], BF16)
            nc.sync.dma_start(wqkv_sb[:], wqkv.rearrange("c p f -> p c f"))
            bqkv_sb = singles.tile([128, NOC], F32)
            nc.sync.dma_start(bqkv_sb[:], bqkv.rearrange("o p one -> p (o one)"))
            eps_sb = singles.tile([128, 1], F32)
            nc.vector.memset(eps_sb[:], EPS)
            bout_sb = singles.tile([1, D], F32R)
            nc.sync.dma_start(bout_sb[:], bout[:])
            ones_sb = singles.tile([1, 128], F32R)
            nc.sync.dma_start(ones_sb[:], ones[:])
            ones_b = singles.tile([1, 128], BF16)
            nc.vector.memset(ones_b[:], 1.0)
            wo_sb = singles.tile([128, ND, D], BF16)
            nc.sync.dma_start(wo_sb[:], wo.rearrange("c p f -> p c f"))
            bin_sb = singles.tile([128, NM], F32)
            nc.sync.dma_start(bin_sb[:], bin_[:])
            # token-major V with a ones column at col DH (softmax denominator);
            # cols 0:DH are overwritten per batch by DMA, col DH stays 1.0.
            # Double-buffered by batch so batch 1's loads overlap batch 0.
            vt = [
                [
                    singles.tile([128, NKC, DH + 1], BF16, name=f"vt{b}{h}")
                    for h in range(HPC)
                ]
                for b in range(B)
            ]
            for b in range(B):
                for h in range(HPC):
                    nc.vector.memset(vt[b][h][:], 1.0)

            for rep in range(reps):
                # ---------- phase A: LN1 + QKV (all heads, my tokens) ----------
              with (
                  tc.tile_pool(name=f"a_x{rep}", bufs=1) as axp,
                  tc.tile_pool(name=f"a_st{rep}", bufs=1) as astp,
                  tc.tile_pool(name=f"a_sm{rep}", bufs=3) as asm,
                  tc.tile_pool(name=f"a_ps{rep}", bufs=2, space="PSUM") as aps,
              ):
                  mvs = astp.tile([128, 4, 2], F32, tag="mvs")
                  xall = axp.tile([128, 4, D], F32, tag="xall")
                  nc.scalar.dma_start(
                      xall[:], resid_mine.rearrange("(t p) d -> p t d", p=128)
                  )
                  for t in range(4):
                      stats = asm.tile([128, 2, 6], F32, tag="stats")
                      nc.vector.bn_stats(stats[:, 0, :], xall[:, t, 0:512])
                      nc.vector.bn_stats(stats[:, 1, :], xall[:, t, 512:1024])
                      nc.vector.bn_aggr(mvs[:, t, :], stats[:])
                  stds = asm.tile([128, 4], F32, tag="stds")
                  nc.scalar.activation(stds[:], mvs[:, :, 1], AF.Sqrt, bias=eps_sb[:])
                  rstds = astp.tile([128, 4], F32, tag="rstds")
                  nc.vector.reciprocal(rstds[:], stds[:])

                  xlnT = astp.tile([128, ND, TSH], BF16, tag="xlnT")
                  for t in range(4):
                      xln = asm.tile([128, D], BF16, tag="xln")
                      nc.vector.tensor_scalar(
                          out=xln[:],
                          in0=xall[:, t, :],
                          scalar1=mvs[:, t, 0:1],
                          scalar2=rstds[:, t : t + 1],
                          op0=OP.subtract,
                          op1=OP.mult,
                      )
                      tpb = aps.tile([128, ND, 128], BF16, tag="tpb")
                      for dc in range(ND):
                          nc.tensor.transpose(
                              tpb[:, dc, :], xln[:, dc * 128 : (dc + 1) * 128], identb[:]
                          )
                      nc.vector.tensor_copy(
                          xlnT[:, :, t * 128 : (t + 1) * 128], tpb[:]
                      )

                  # QKV for all heads over my 512 tokens; stage for the A2A.
                  qkstage = astp.tile([128, 2 * N_CORES, TSH], BF16, tag="qkstage")
                  vstage = astp.tile([128, N_CORES, 4, 128], BF16, tag="vstage")
                  for j in range(N_CORES):
                      for kind in range(3):
                          oc = 3 * j + kind
                          ps = aps.tile([128, 512], F32, tag="qkvps")
                          for dc in range(ND):
                              nc.tensor.matmul(
                                  ps[:],
                                  wqkv_sb[:, dc, oc * 128 : (oc + 1) * 128],
                                  xlnT[:, dc, :],
                                  start=(dc == 0),
                                  stop=(dc == ND - 1),
                              )
                          if kind < 2:
                              nc.vector.tensor_scalar_add(
                                  out=qkstage[:, 2 * j + kind, :],
                                  in0=ps[:],
                                  scalar1=bqkv_sb[:, oc : oc + 1],
                              )
                          else:
                              vsb = asm.tile([128, 512], BF16, tag="vsb")
                              nc.vector.tensor_scalar_add(
                                  out=vsb[:], in0=ps[:], scalar1=bqkv_sb[:, oc : oc + 1]
                              )
                              vtp = aps.tile([128, 4, 128], BF16, tag="vtp")
                              for tb in range(4):
                                  nc.tensor.transpose(
                                      vtp[:, tb, :],
                                      vsb[:, tb * 128 : (tb + 1) * 128],
                                      identb[:],
                                  )
                              nc.vector.tensor_copy(vstage[:, j, :, :], vtp[:])
                  for b in range(B):
                      # q/k: one DMA per (batch, kind) covering all 8 dsts
                      for kind in range(2):
                          nc.sync.dma_start(
                              qkv_in[b][:, kind * 128 : (kind + 1) * 128, :]
                              .rearrange("j p c -> p j c"),
                              qkstage[:, :, b * 256 : (b + 1) * 256]
                              .rearrange("p (j k) c -> p j k c", k=2)[:, :, kind, :],
                          )
                      # v (token-major packing): one DMA per batch
                      nc.sync.dma_start(
                          qkv_in[b][:, 256:384, :].rearrange(
                              "j p (t f) -> p j t f", t=2
                          ),
                          vstage[:, :, b * 2 : (b + 1) * 2, :],
                      )
                  for b in range(B):
                      if FAKE_A2A:
                          nc.sync.dma_start(qkv_out[b][:], qkv_in[b][:])
                      else:
                          nc.gpsimd.collective_compute(
                              "AllToAll",
                              OP.bypass,
                              replica_groups=[list(range(N_CORES))],
                              ins=[qkv_in[b][:]],
                              outs=[qkv_out[b][:]],
                          )

              # ---------- phase B: attention (my 2 heads, full sequence) ----------
              with (
                  tc.tile_pool(name=f"b_qk{rep}", bufs=2, side="right") as bqk,
                  tc.tile_pool(name=f"b_sm{rep}", bufs=4, side="right") as bsm,
                  tc.tile_pool(name=f"b_ps{rep}", bufs=2, space="PSUM") as bps,
              ):
                  for b in range(B):
                      qT = bqk.tile([128, S], BF16, tag="qT", name=f"qT{b}")
                      kT = bqk.tile([128, S], BF16, tag="kT", name=f"kT{b}")
                      nc.sync.dma_start(
                          qT[:].rearrange("p (i c) -> p i c", i=N_CORES),
                          qkv_out[b][:, 0:128, :].rearrange("i p c -> p i c"),
                      )
                      nc.sync.dma_start(
                          kT[:].rearrange("p (i c) -> p i c", i=N_CORES),
                          qkv_out[b][:, 128:256, :].rearrange("i p c -> p i c"),
                      )
                      vload = bqk.tile([128, N_CORES, 256], BF16, tag="vload")
                      nc.sync.dma_start(
                          vload[:],
                          qkv_out[b][:, 256:384, :].rearrange("i p c -> p i c"),
                      )
                      for h in range(HPC):
                          nc.vector.tensor_copy(
                              vt[b][h][:, :, 0:DH].rearrange(
                                  "p (i t) f -> p i t f", t=2
                              ),
                              vload[:].rearrange("p i (t f) -> p i t f", t=2)[
                                  :, :, :, h * DH : (h + 1) * DH
                              ],
                          )
                      znall = bqk.tile([128, NQC, 512], BF16, tag="znall", name=f"zn{b}")
                      for qc in range(NQC):
                          nkc = 4 * qc + 4
                          hss = [slice(h * DH, (h + 1) * DH) for h in range(HPC)]
                          zps = [
                              bps.tile([DH + 1, 512], F32, tag="zpsum", name=f"zp{h}")
                              for h in range(HPC)
                          ]
                          for kc in range(nkc):
                              sp2 = bps.tile([128, 1024], F32, tag="sp2", bufs=3)
                              for h in range(HPC):
                                  nc.tensor.matmul(
                                      sp2[:, h * 512 : (h + 1) * 512],
                                      kT[hss[h], kc * 128 : (kc + 1) * 128],
                                      qT[hss[h], qc * 512 : (qc + 1) * 512],
                                      start=True,
                                      stop=True,
                                  )
                              es2 = bsm.tile([128, 1024], BF16, tag="es2", bufs=6)
                              p = kc - 4 * qc  # diagonal-band offset if >= 0
                              if p > 0:
                                  # columns [0:128p] of each half are fully
                                  # causally masked: zero them, skip the exp
                                  nc.vector.memset(
                                      es2[:].rearrange("x (h c) -> x h c", h=2)[
                                          :, :, 0 : 128 * p
                                      ],
                                      0.0,
                                  )
                                  nc.scalar.activation(
                                      es2[:].rearrange("x (h c) -> x h c", h=2)[
                                          :, :, 128 * p : 512
                                      ],
                                      sp2[:].rearrange("x (h c) -> x h c", h=2)[
                                          :, :, 128 * p : 512
                                      ],
                                      AF.Exp,
                                  )
                              else:
                                  nc.scalar.activation(es2[:], sp2[:], AF.Exp)
                              if p >= 0:
                                  # triangular band: cols [128p : 128p+128]
                                  band = es2[:].rearrange("x (h c) -> x h c", h=2)[
                                      :, :, 128 * p : 128 * p + 128
                                  ]
                                  nc.vector.tensor_tensor(
                                      band,
                                      band,
                                      mask_sb[:].rearrange("x (h c) -> x h c", h=2),
                                      OP.mult,
                                  )
                              for h in range(HPC):
                                  nc.tensor.matmul(
                                      zps[h][:],
                                      vt[b][h][:, kc, :],
                                      es2[:, h * 512 : (h + 1) * 512],
                                      start=(kc == 0),
                                      stop=(kc == nkc - 1),
                                  )
                          bct = bps.tile([128, 1024], F32, tag="sp2", bufs=3, name="bct")
                          for h in range(HPC):
                              recip = bsm.tile([1, 512], BF16, tag="recip")
                              with nc.allow_low_precision(
                                  reason="bf16 softmax denom; ~0.4% on z, tiny abs"
                              ):
                                  nc.vector.reciprocal(recip[:], zps[h][DH : DH + 1, :])
                              nc.tensor.matmul(
                                  bct[h * DH : (h + 1) * DH, 0:512],
                                  ones_b[:, 0:DH],
                                  recip[:],
                                  start=True,
                                  stop=True,
                              )
                              bcr = bsm.tile([DH, 512], F32, tag="bcr")
                              nc.vector.tensor_copy(bcr[:], bct[h * DH : (h + 1) * DH, 0:512])
                              with nc.allow_low_precision(
                                  reason="bf16 z for the A2A wire; feeds only o-proj"
                              ):
                                  nc.vector.tensor_tensor(
                                      znall[h * DH : (h + 1) * DH, qc, :],
                                      zps[h][0:DH, :],
                                      bcr[:],
                                      OP.mult,
                                  )
                      nc.sync.dma_start(
                          z_in[b][:].rearrange("(q s) hp c -> hp q s c", s=2),
                          znall[:].rearrange("hp q (s c) -> hp q s c", s=2),
                      )
                      if FAKE_A2A:
                          nc.sync.dma_start(z_out[b][:], z_in[b][:])
                      else:
                          nc.gpsimd.collective_compute(
                              "AllToAll",
                              OP.bypass,
                              replica_groups=[list(range(N_CORES))],
                              ins=[z_in[b][:]],
                              outs=[z_out[b][:]],
                          )

              # ---------------- phase C: post (token-parallel) ----------------
              with (
                  tc.tile_pool(name=f"post_w{rep}", bufs=4, side="right") as pw,
                  tc.tile_pool(name=f"post_big{rep}", bufs=1, side="right") as pbig,
                  tc.tile_pool(name=f"post_t{rep}", bufs=3, side="right") as pt,
                  tc.tile_pool(name=f"post_ps{rep}", bufs=2, space="PSUM") as pps,
              ):
                  resid2 = pbig.tile([128, 4, D], F32, tag="resid2")

                  # broadcast b_out across partitions once: [1, D] -> [128, D]
                  bout_full = pbig.tile([128, D], F32, tag="bout_full")
                  for dc2 in range(2):
                      bps_ = pps.tile([128, 512], F32, tag="ps1")
                      nc.tensor.matmul(
                          bps_[:],
                          ones_sb[:],
                          bout_sb[:, dc2 * 512 : (dc2 + 1) * 512],
                          start=True,
                          stop=True,
                      )
                      nc.vector.tensor_copy(bout_full[:, dc2 * 512 : (dc2 + 1) * 512], bps_[:])

                  # o-projection + residual add -> resid2
                  with tc.tile_pool(name=f"post_o{rep}", bufs=1) as po:
                      zt = po.tile([128, N_CORES, 512], BF16, tag="zt")
                      for b in range(B):
                          nc.scalar.dma_start(
                              zt[:, :, b * 256 : (b + 1) * 256],
                              z_out[b][:].rearrange("i p c -> p i c"),
                          )
                      rsall = pbig.tile([128, 4, D], F32, tag="rsall")
                      nc.scalar.dma_start(
                          rsall[:], resid_mine.rearrange("(t p) d -> p t d", p=128)
                      )
                      for tsub in range(4):
                          rs = rsall[:, tsub, :]
                          for dc2 in range(2):
                              op_ = pps.tile([128, 512], F32, tag="ps1")
                              for hd in range(ND):
                                  nc.tensor.matmul(
                                      op_[:],
                                      zt[:, hd, tsub * 128 : (tsub + 1) * 128],
                                      wo_sb[:, hd, dc2 * 512 : (dc2 + 1) * 512],
                                      start=(hd == 0),
                                      stop=(hd == ND - 1),
                                  )
                              nc.vector.tensor_tensor(
                                  resid2[:, tsub, dc2 * 512 : (dc2 + 1) * 512],
                                  op_[:],
                                  rsall[:, tsub, dc2 * 512 : (dc2 + 1) * 512],
                                  OP.add,
                              )

                  # LN2 + transpose -> xln2T [128, ND, 512] (one batched Sqrt)
                  xln2T = pbig.tile([128, ND, 512], FP8, tag="xln2T")
                  mvs2 = pbig.tile([128, 4, 2], F32, tag="mvs2")
                  for tsub in range(4):
                      stats = pt.tile([128, 2, 6], F32, tag="stats2")
                      nc.vector.bn_stats(stats[:, 0, :], resid2[:, tsub, 0:512])
                      nc.vector.bn_stats(stats[:, 1, :], resid2[:, tsub, 512:1024])
                      nc.vector.bn_aggr(mvs2[:, tsub, :], stats[:])
                  stds2 = pbig.tile([128, 4], F32, tag="stds2")
                  nc.scalar.activation(stds2[:], mvs2[:, :, 1], AF.Sqrt, bias=eps_sb[:])
                  rstds2 = pbig.tile([128, 4], F32, tag="rstds2")
                  nc.vector.reciprocal(rstds2[:], stds2[:])
                  for tsub in range(4):
                      xln2 = pt.tile([128, D], BF16, tag="xln2")
                      nc.vector.tensor_scalar(
                          out=xln2[:],
                          in0=resid2[:, tsub, :],
                          scalar1=mvs2[:, tsub, 0:1],
                          scalar2=rstds2[:, tsub : tsub + 1],
                          op0=OP.subtract,
                          op1=OP.mult,
                      )
                      tpb = pps.tile([128, ND, 128], BF16, tag="ps1t")
                      for dc in range(ND):
                          nc.tensor.transpose(
                              tpb[:, dc, :], xln2[:, dc * 128 : (dc + 1) * 128], identb[:]
                          )
                      nc.vector.tensor_copy(
                          xln2T[:, :, tsub * 128 : (tsub + 1) * 128], tpb[:]
                      )

                  # MLP pass A: h1^T per m-chunk -> gelu -> gT; accumulate out d 0:512
                  gT = pbig.tile([128, NM, 512], FP8, tag="gT")
                  acc = pps.tile([128, 4, 512], F32, tag="acc", bufs=1)
                  for mq in range(NM // 4):
                      wi = pw.tile([128, ND, 512], FP8, tag="wi")
                      nc.sync.dma_start(
                          wi[:], win[mq].rearrange("c p f -> p c f")
                      )
                      wu = pw.tile([128, 4, 512], FP8, tag="wu")
                      nc.sync.dma_start(
                          wu[:],
                          wout[4 * mq : 4 * mq + 4, :, 0:512].rearrange(
                              "m p f -> p m f"
                          ),
                      )
                      for mi in range(4):
                          m = 4 * mq + mi
                          h1 = pps.tile([128, 512], F32, tag="ps1")
                          for dcp in range(ND // 2):
                              nc.tensor.matmul(
                                  h1[:],
                                  wi[:, 2 * dcp : 2 * dcp + 2, mi * 128 : (mi + 1) * 128],
                                  xln2T[:, 2 * dcp : 2 * dcp + 2, :],
                                  start=(dcp == 0),
                                  stop=(dcp == ND // 2 - 1),
                                  perf_mode=mybir.MatmulPerfMode.DoubleRow,
                              )
                          with nc.allow_low_precision(
                              reason="fp8 MLP activations; ~8.5e-3 rel err measured"
                          ):
                              nc.scalar.activation(
                                  gT[:, m, :], h1[:], AF.Gelu_apprx_tanh,
                                  bias=bin_sb[:, m : m + 1],
                                  scale=1.0 / WS_MLP,
                              )
                      for mi2 in range(2):
                          for tsub in range(4):
                              nc.tensor.matmul(
                                  acc[:, tsub, :],
                                  gT[
                                      :,
                                      4 * mq + 2 * mi2 : 4 * mq + 2 * mi2 + 2,
                                      tsub * 128 : (tsub + 1) * 128,
                                  ],
                                  wu[:, 2 * mi2 : 2 * mi2 + 2, :],
                                  start=(mq == 0 and mi2 == 0),
                                  stop=(mq == NM // 4 - 1 and mi2 == 1),
                                  perf_mode=mybir.MatmulPerfMode.DoubleRow,
                              )
                  otall = pbig.tile([128, 4, 512], F32, tag="otall")
                  for tsub in range(4):
                      nc.vector.scalar_tensor_tensor(
                          otall[:, tsub, :], acc[:, tsub, :], 1.0 / WS_MLP,
                          resid2[:, tsub, 0:512], OP.mult, OP.add,
                      )
                      nc.vector.tensor_tensor(
                          otall[:, tsub, :], otall[:, tsub, :], bout_full[:, 0:512], OP.add
                      )
                  nc.scalar.dma_start(
                      y[:, 0:512].rearrange("(t p) f -> p t f", p=128), otall[:]
                  )

                  # MLP pass B: same gT, out d 512:1024
                  acc2 = pps.tile([128, 4, 512], F32, tag="acc", bufs=1)
                  for mq in range(NM // 4):
                      wu = pw.tile([128, 4, 512], FP8, tag="wu")
                      nc.sync.dma_start(
                          wu[:],
                          wout[4 * mq : 4 * mq + 4, :, 512:1024].rearrange(
                              "m p f -> p m f"
                          ),
                      )
                      for mi2 in range(2):
                          for tsub in range(4):
                              nc.tensor.matmul(
                                  acc2[:, tsub, :],
                                  gT[
                                      :,
                                      4 * mq + 2 * mi2 : 4 * mq + 2 * mi2 + 2,
                                      tsub * 128 : (tsub + 1) * 128,
                                  ],
                                  wu[:, 2 * mi2 : 2 * mi2 + 2, :],
                                  start=(mq == 0 and mi2 == 0),
                                  stop=(mq == NM // 4 - 1 and mi2 == 1),
                                  perf_mode=mybir.MatmulPerfMode.DoubleRow,
                              )
                  otall2 = pbig.tile([128, 4, 512], F32, tag="otall2")
                  for tsub in range(4):
                      nc.vector.scalar_tensor_tensor(
                          otall2[:, tsub, :], acc2[:, tsub, :], 1.0 / WS_MLP,
                          resid2[:, tsub, 512:1024], OP.mult, OP.add,
                      )
                      nc.vector.tensor_tensor(
                          otall2[:, tsub, :], otall2[:, tsub, :], bout_full[:, 512:1024], OP.add
                      )
                  nc.scalar.dma_start(
                      y[:, 512:1024].rearrange("(t p) f -> p t f", p=128), otall2[:]
                  )

    nc.compile()
    return nc


def _prep_inputs(inputs):
    """Host-side weight folding; returns per-core in_maps."""
    f32 = np.float32
    resid = np.asarray(inputs["resid"], f32)
    w_q = np.asarray(inputs["w_q"], f32)
    w_k = np.asarray(inputs["w_k"], f32)
    w_v = np.asarray(inputs["w_v"], f32)
    w_o = np.asarray(inputs["w_o"], f32)
    ln1_w = np.asarray(inputs["ln1_w"], f32)
    ln1_b = np.asarray(inputs["ln1_b"], f32)
    ln2_w = np.asarray(inputs["ln2_w"], f32)
    ln2_b = np.asarray(inputs["ln2_b"], f32)
    w_in = np.asarray(inputs["w_in"], f32)
    b_in = np.asarray(inputs["b_in"], f32)
    w_out = np.asarray(inputs["w_out"], f32)
    b_out = np.asarray(inputs["b_out"], f32)

    sm = 1.0 / np.sqrt(DH)
    win_f = ln2_w[:, None] * w_in  # [D, DM]
    bin_f = ln2_b @ w_in + b_in  # [DM]

    # [NM//2, ND, 128, 256]: pair m-chunks (2mp, 2mp+1) side by side in f;
    # scaled by WS_MLP so fp8e4m3 sees a good exponent range (undone in gelu)
    win_host = np.ascontiguousarray(
        (win_f * WS_MLP)
        .reshape(ND, 128, NM // 4, 4, 128)
        .transpose(2, 0, 1, 3, 4)
        .reshape(NM // 4, ND, 128, 512)
        .astype(ml_dtypes.float8_e4m3)
    )
    # bin layout [128, NM]: column m holds, on partition p, the bias of
    # m-feature 128*m + p (gelu bias is per-partition of the h1 psum tile).
    bin_host = np.ascontiguousarray(bin_f.reshape(NM, 128).T)
    wout_host = np.ascontiguousarray(
        (w_out * WS_MLP).reshape(NM, 128, D).astype(ml_dtypes.float8_e4m3)
    )
    wo_host = np.ascontiguousarray(
        w_o.reshape(H * DH, D).reshape(ND, 128, D).astype(ml_dtypes.bfloat16)
    )
    bout_host = np.ascontiguousarray(b_out.reshape(1, D))

    ones_host = np.ones((1, 128), f32)

    # QKV weights, dst-major: for dst core j: q(heads 2j,2j+1) | k | v, 128
    # cols each. LN1 gain and the softmax scale are folded in.
    wq_f = (ln1_w[:, None, None] * w_q.transpose(1, 0, 2) * sm).reshape(D, D)
    wk_f = (ln1_w[:, None, None] * w_k.transpose(1, 0, 2)).reshape(D, D)
    wv_f = (ln1_w[:, None, None] * w_v.transpose(1, 0, 2)).reshape(D, D)
    bq_f = (ln1_b @ w_q.transpose(1, 0, 2).reshape(D, D)) * sm
    bk_f = ln1_b @ w_k.transpose(1, 0, 2).reshape(D, D)
    bv_f = ln1_b @ w_v.transpose(1, 0, 2).reshape(D, D)
    wcols = []
    bcols = []
    for j in range(N_CORES):
        fs = slice(j * 128, (j + 1) * 128)
        wcols += [wq_f[:, fs], wk_f[:, fs], wv_f[:, fs]]
        bcols += [bq_f[fs], bk_f[fs], bv_f[fs]]
    wqkv_host = np.ascontiguousarray(
        np.concatenate(wcols, axis=1).reshape(ND, 128, 3 * D).astype(ml_dtypes.bfloat16)
    )
    bqkv_host = np.ascontiguousarray(np.stack(bcols).reshape(NOC, 128, 1))

    tri = (np.arange(128)[:, None] <= np.arange(128)[None, :]).astype(
        ml_dtypes.bfloat16
    )
    masks_host = np.ascontiguousarray(np.concatenate([tri, tri], axis=1))
    ident_host = np.eye(128, dtype=f32)

    in_maps = []
    for c in range(N_CORES):
        t0 = TPB * c
        rm = np.concatenate(
            [resid[0, t0 : t0 + TPB], resid[1, t0 : t0 + TPB]], axis=0
        )
        in_maps.append(
            {
                "resid_mine": np.ascontiguousarray(rm),
                "wqkv": wqkv_host,
                "bqkv": bqkv_host,
                "wo": wo_host,
                "win": win_host,
                "bin": bin_host,
                "wout": wout_host,
                "bout": bout_host,
                "masks": masks_host,
                "ident": ident_host,
                "ones": ones_host,
            }
        )
    return in_maps


class _Runner:
    """Compile once; keep the jitted shard_map callable and device-resident
    inputs so repeat executes measure the kernel, not host overhead."""

    def __init__(self):
        import jax
        from concourse import bass2jax

        self.jax = jax
        self.bass2jax = bass2jax
        bass2jax.install_neuronx_cc_hook()
        nc = build_nc()
        self.nc = nc

        in_names, out_names, out_avals, zero_shapes = [], [], [], []
        for alloc in nc.m.functions[0].allocations:
            if not isinstance(alloc, mybir.MemoryLocationSet):
                continue
            name = alloc.memorylocations[0].name
            if alloc.kind == "ExternalInput":
                if not (nc.partition_id_tensor and name == nc.partition_id_tensor.name):
                    in_names.append(name)
            elif alloc.kind == "ExternalOutput":
                shape = tuple(alloc.tensor_shape)
                dtype = mybir.dt.np(alloc.dtype)
                out_names.append(name)
                out_avals.append(jax.core.ShapedArray(shape, dtype))
                zero_shapes.append((shape, dtype))
        n_params = len(in_names)
        all_in_names = list(in_names) + list(out_names)
        partition_name = (
            nc.partition_id_tensor.name if nc.partition_id_tensor else None
        )
        if partition_name is not None:
            all_in_names.append(partition_name)
        self.in_names = in_names
        self.out_names = out_names
        self.zero_shapes = zero_shapes
        n_outs = len(out_names)

        def _body(*args):
            operands = list(args)
            if partition_name is not None:
                operands.append(bass2jax.partition_id_tensor())
            outs = bass2jax._bass_exec_p.bind(
                *operands,
                out_avals=tuple(out_avals),
                in_names=tuple(all_in_names),
                out_names=tuple(out_names),
                lowering_input_output_aliases=(),
                sim_require_finite=True,
                sim_require_nnan=True,
                nc=nc,
            )
            return tuple(outs)

        from jax.sharding import Mesh, NamedSharding, PartitionSpec
        from jax.experimental.shard_map import shard_map

        devices = jax.devices()[:N_CORES]
        self.mesh = Mesh(np.asarray(devices), ("core",))
        self.sharding = NamedSharding(self.mesh, PartitionSpec("core"))
        donate = tuple(range(n_params, n_params + n_outs))
        in_specs = (PartitionSpec("core"),) * (n_params + n_outs)
        out_specs = (PartitionSpec("core"),) * n_outs
        self.sharded = jax.jit(
            shard_map(
                _body,
                mesh=self.mesh,
                in_specs=in_specs,
                out_specs=out_specs,
                check_rep=False,
            ),
            donate_argnums=donate,
            keep_unused=True,
        )

    def put_inputs(self, in_maps):
        concat = [
            np.concatenate([np.asarray(m[name]) for m in in_maps], axis=0)
            for name in self.in_names
        ]
        return [self.jax.device_put(a, self.sharding) for a in concat]

    def _zeros(self):
        return [
            np.zeros((N_CORES * s[0], *s[1:]), dt) for (s, dt) in self.zero_shapes
        ]

    def execute(self, dev_in):
        outs = self.sharded(*dev_in, *self._zeros())
        for o in outs:
            o.block_until_ready()
        return outs

    def gather(self, outs):
        per_core = {}
        for i, name in enumerate(self.out_names):
            arr = np.asarray(outs[i])
            per_core[name] = arr.reshape(N_CORES, -1, *arr.shape[1:])
        return per_core


_RUNNER = None


def _get_runner():
    global _RUNNER
    if _RUNNER is None:
        _RUNNER = _Runner()
    return _RUNNER


def kernel(**inputs) -> np.ndarray:
    r = _get_runner()
    in_maps = _prep_inputs(inputs)
    dev_in = r.put_inputs(in_maps)
    outs = r.execute(dev_in)
    ys = r.gather(outs)["y"]  # [8, 512, 1024]
    out = np.zeros((B, S, D), np.float32)
    for c in range(N_CORES):
        out[0, TPB * c : TPB * c + TPB] = ys[c][0:TPB]
        out[1, TPB * c : TPB * c + TPB] = ys[c][TPB : 2 * TPB]
    return out


if __name__ == "__main__":
    # quick self-exercise with random data
    rng = np.random.default_rng(0)
    ins = {
        "resid": rng.standard_normal((B, S, D)).astype(np.float32),
        "w_q": 0.02 * rng.standard_normal((H, D, DH)).astype(np.float32),
        "w_k": 0.02 * rng.standard_normal((H, D, DH)).astype(np.float32),
        "w_v": 0.02 * rng.standard_normal((H, D, DH)).astype(np.float32),
        "w_o": 0.02 * rng.standard_normal((H, DH, D)).astype(np.float32),
        "ln1_w": 0.02 * rng.standard_normal(D).astype(np.float32),
        "ln1_b": np.zeros(D, np.float32),
        "ln2_w": 0.02 * rng.standard_normal(D).astype(np.float32),
        "ln2_b": np.zeros(D, np.float32),
        "w_in": 0.02 * rng.standard_normal((D, DM)).astype(np.float32),
        "b_in": np.zeros(DM, np.float32),
        "w_out": 0.02 * rng.standard_normal((DM, D)).astype(np.float32),
        "b_out": np.zeros(D, np.float32),
    }
    out = kernel(**ins)
    print("out", out.shape, out.dtype, float(np.abs(out).mean()))


# revision 27
# speedup vs baseline: 1.2451x; 1.2451x over previous
"""Trainium2 Bass kernel for a dense transformer block (nn_Block_29583734734992).

Reference computation (fp32):
    resid = resid + Attn(LN1(resid))          # 16 heads, d_head 64, causal
    resid = resid + MLP(LN2(resid)) + b_out   # d_mlp 4096, tanh-gelu

Sharding over 8 NeuronCores (v2 — token-sharded LN/QKV + head-sharded attention):
  - Phase A (token-parallel): core c owns 512 tokens (rows [256c, 256c+256) of
    each batch). It LN1s + transposes ONLY those tokens and computes the QKV
    projections for ALL 16 heads on them (same total FLOPs as head-parallel
    QKV, but the LN/transpose work is sharded 8x instead of replicated 8x).
    V is pre-transposed to token-major on the sender. One AllToAll per batch
    reshards q/k/v to head-owners.
  - Phase B (head-parallel): core c holds heads (2c, 2c+1) with full-sequence
    qT/kT (feature-major) and token-major V. Causal scores/softmax/z as in
    v1, but the two heads' score tiles share one 2-bank PSUM tile so a single
    ACT exp covers both ([128,1024] per key-chunk). Two AllToAlls (one per
    batch) reshard z back to token-owners; each fires as soon as that batch's
    attention is staged so it hides under the other batch / post compute.
  - Phase C (token-parallel): o-projection, residual add, LN2 and the full
    MLP for the core's 512 tokens, writing a [512, 1024] output shard.

Numerics: bf16 matmuls with fp32 PSUM accumulation throughout; LN scale/bias,
the 1/sqrt(64) softmax scale and b_in are folded into weights / activation
biases on the host. Softmax skips max-subtraction (scores are small) and
applies the causal mask multiplicatively after exp; the per-query softmax
denominator comes from an extra ones-column appended to V's stationary
operand. DMAs are batched into multi-dim access patterns (each dma_start
costs ~0.7us of Sync-queue issue time).
"""

import sys

for _p in ("/opt/trn_rl_repo", "/root/.axon_site/_ro/trn_rl_repo"):
    if _p not in sys.path:
        sys.path.insert(0, _p)

import ml_dtypes
import numpy as np

import concourse.bass as bass
import concourse.mybir as mybir
import concourse.tile as tile
from concourse import bacc
from concourse.bass_utils import run_bass_kernel_spmd

F32 = mybir.dt.float32
F32R = mybir.dt.float32r
BF16 = mybir.dt.bfloat16
FP8 = mybir.dt.float8e4
WS_MLP = 64.0  # host scale on w_in/w_out so fp8e4m3 sees a good exponent range
# MLP matmul precision: fp8e4m3 + DoubleRow (2 contraction chunks per MM) vs
# plain bf16. Toggle for A/B timing; numerics pass the gate either way.
MLP_FP8 = True
AF = mybir.ActivationFunctionType
OP = mybir.AluOpType

N_CORES = 8
B, S, D = 2, 2048, 1024
H, DH, DM = 16, 64, 4096
EPS = 1e-5
HPC = H // N_CORES  # heads per core = 2
TSH = (B * S) // N_CORES  # tokens per core = 512 (256 from each batch)
TPB = TSH // B  # tokens per core per batch = 256
ND = D // 128  # 8 d_model chunks
NM = DM // 128  # 32 d_mlp chunks
NQC = S // 512  # 4 query chunks of 512
NKC = S // 128  # 16 key chunks of 128
NOC = 3 * N_CORES  # 24 QKV output chunks of 128 (dst-major: q,k,v per dst)

# Replace the A2A collectives with local DRAM copies so the module has no
# collectives (lets TimelineSim model a single core). Timing-analysis only.
FAKE_A2A = False
# Merge the per-batch A2As into one collective per direction (2 sync points
# per rep instead of 4); trades batch-level overlap for fewer barriers.
MERGE_A2A = False


def build_nc(reps: int = 1):
    nc = bacc.Bacc(
        "TRN2",
        target_bir_lowering=False,
        debug=False,
        num_devices=1 if FAKE_A2A is True else N_CORES,
    )

    resid_mine = nc.dram_tensor("resid_mine", [TSH, D], F32, kind="ExternalInput")
    wqkv = nc.dram_tensor("wqkv", [ND, 128, 3 * D], BF16, kind="ExternalInput")
    bqkv = nc.dram_tensor("bqkv", [NOC, 128, 1], F32, kind="ExternalInput")
    wo = nc.dram_tensor("wo", [ND, 128, D], BF16, kind="ExternalInput")
    MDT = FP8 if MLP_FP8 else BF16
    win = nc.dram_tensor("win", [NM // 4, ND, 128, 512], MDT, kind="ExternalInput")
    bin_ = nc.dram_tensor("bin", [128, NM], F32, kind="ExternalInput")
    wout = nc.dram_tensor("wout", [NM, 128, D], MDT, kind="ExternalInput")
    bout = nc.dram_tensor("bout", [1, D], F32R, kind="ExternalInput")
    masks = nc.dram_tensor("masks", [128, 256], BF16, kind="ExternalInput")
    ident = nc.dram_tensor("ident", [128, 128], F32, kind="ExternalInput")
    ones = nc.dram_tensor("ones", [1, 128], F32R, kind="ExternalInput")
    y = nc.dram_tensor("y", [TSH, D], F32, kind="ExternalOutput")

    with tile.TileContext(nc) as tc:
        with (
            tc.tile_pool(name="singles", bufs=1) as singles,
            tc.tile_pool(name="dram", bufs=1, space="DRAM") as dram,
        ):
            # A2A buffers. qkv block per dst j: rows 0:128 q feats (heads
            # 2j,2j+1), 128:256 k feats, 256:384 v in token-major packing
            # (row r, col tb*128+f  ->  v[token tb*128+r, feat f]).
            if MERGE_A2A:
                qkv_in_m = dram.tile(
                    [N_CORES, B * 3 * 128, TPB], BF16, tag="qim", name="qim"
                )
                qkv_out_m = dram.tile(
                    [N_CORES, B * 3 * 128, TPB], BF16, tag="qom", name="qom"
                )
                z_in_m = dram.tile(
                    [N_CORES, B * HPC * DH, TPB], BF16, tag="zim", name="zim"
                )
                z_out_m = dram.tile(
                    [N_CORES, B * HPC * DH, TPB], BF16, tag="zom", name="zom"
                )
                qkv_in = [qkv_in_m[:, b * 384 : (b + 1) * 384, :] for b in range(B)]
                qkv_out = [qkv_out_m[:, b * 384 : (b + 1) * 384, :] for b in range(B)]
                z_in = [z_in_m[:, b * 128 : (b + 1) * 128, :] for b in range(B)]
                z_out = [z_out_m[:, b * 128 : (b + 1) * 128, :] for b in range(B)]
            else:
                qkv_in = [
                    dram.tile([N_CORES, 3 * 128, TPB], BF16, tag=f"qi{b}", name=f"qi{b}")
                    for b in range(B)
                ]
                qkv_out = [
                    dram.tile([N_CORES, 3 * 128, TPB], BF16, tag=f"qo{b}", name=f"qo{b}")
                    for b in range(B)
                ]
                z_in = [
                    dram.tile([N_CORES, HPC * DH, TPB], BF16, tag=f"zi{b}", name=f"zi{b}")
                    for b in range(B)
                ]
                z_out = [
                    dram.tile([N_CORES, HPC * DH, TPB], BF16, tag=f"zo{b}", name=f"zo{b}")
                    for b in range(B)
                ]

            ident_sb = singles.tile([128, 128], F32)
            nc.sync.dma_start(ident_sb[:], ident[:])
            identb = singles.tile([128, 128], BF16)
            nc.vector.tensor_copy(identb[:], ident_sb[:])
            mask_sb = singles.tile([128, 256], BF16)
            nc.sync.dma_start(mask_sb[:], masks[:])
            wqkv_sb = singles.tile([128, ND, 3 * D], BF16)
            nc.sync.dma_start(wqkv_sb[:], wqkv.rearrange("c p f -> p c f"))
            bqkv_sb = singles.tile([128, NOC], F32)
            nc.sync.dma_start(bqkv_sb[:], bqkv.rearrange("o p one -> p (o one)"))
            eps_sb = singles.tile([128, 1], F32)
            nc.vector.memset(eps_sb[:], EPS)
            bout_sb = singles.tile([1, D], F32R)
            nc.sync.dma_start(bout_sb[:], bout[:])
            ones_sb = singles.tile([1, 128], F32R)
            nc.sync.dma_start(ones_sb[:], ones[:])
            ones_b = singles.tile([1, 128], BF16)
            nc.vector.memset(ones_b[:], 1.0)
            wo_sb = singles.tile([128, ND, D], BF16)
            nc.sync.dma_start(wo_sb[:], wo.rearrange("c p f -> p c f"))
            bin_sb = singles.tile([128, NM], F32)
            nc.sync.dma_start(bin_sb[:], bin_[:])
            # token-major V with a ones column at col DH (softmax denominator);
            # cols 0:DH are overwritten per batch by DMA, col DH stays 1.0.
            # Double-buffered by batch so batch 1's loads overlap batch 0.
            vt = [
                [
                    singles.tile([128, NKC, DH + 1], BF16, name=f"vt{b}{h}")
                    for h in range(HPC)
                ]
                for b in range(B)
            ]
            for b in range(B):
                for h in range(HPC):
                    nc.vector.memset(vt[b][h][:], 1.0)

            for rep in range(reps):
                # ---------- phase A: LN1 + QKV (all heads, my tokens) ----------
              with (
                  tc.tile_pool(name=f"a_x{rep}", bufs=1) as axp,
                  tc.tile_pool(name=f"a_st{rep}", bufs=1) as astp,
                  tc.tile_pool(name=f"a_sm{rep}", bufs=3) as asm,
                  tc.tile_pool(name=f"a_ps{rep}", bufs=2, space="PSUM") as aps,
              ):
                  mvs = astp.tile([128, 4, 2], F32, tag="mvs")
                  xall = axp.tile([128, 4, D], F32, tag="xall")
                  nc.scalar.dma_start(
                      xall[:], resid_mine.rearrange("(t p) d -> p t d", p=128)
                  )
                  for t in range(4):
                      stats = asm.tile([128, 2, 6], F32, tag="stats")
                      nc.vector.bn_stats(stats[:, 0, :], xall[:, t, 0:512])
                      nc.vector.bn_stats(stats[:, 1, :], xall[:, t, 512:1024])
                      nc.vector.bn_aggr(mvs[:, t, :], stats[:])
                  stds = asm.tile([128, 4], F32, tag="stds")
                  nc.scalar.activation(stds[:], mvs[:, :, 1], AF.Sqrt, bias=eps_sb[:])
                  rstds = astp.tile([128, 4], F32, tag="rstds")
                  nc.vector.reciprocal(rstds[:], stds[:])

                  xlnT = astp.tile([128, ND, TSH], BF16, tag="xlnT")
                  for t in range(4):
                      xln = asm.tile([128, D], BF16, tag="xln")
                      nc.vector.tensor_scalar(
                          out=xln[:],
                          in0=xall[:, t, :],
                          scalar1=mvs[:, t, 0:1],
                          scalar2=rstds[:, t : t + 1],
                          op0=OP.subtract,
                          op1=OP.mult,
                      )
                      tpb = aps.tile([128, ND, 128], BF16, tag="tpb")
                      for dc in range(ND):
                          nc.tensor.transpose(
                              tpb[:, dc, :], xln[:, dc * 128 : (dc + 1) * 128], identb[:]
                          )
                      nc.vector.tensor_copy(
                          xlnT[:, :, t * 128 : (t + 1) * 128], tpb[:]
                      )

                  # QKV for all heads over my 512 tokens; stage for the A2A.
                  qkstage = astp.tile([128, 2 * N_CORES, TSH], BF16, tag="qkstage")
                  vstage = astp.tile([128, N_CORES, 4, 128], BF16, tag="vstage")
                  for j in range(N_CORES):
                      for kind in range(3):
                          oc = 3 * j + kind
                          ps = aps.tile([128, 512], F32, tag="qkvps")
                          for dc in range(ND):
                              nc.tensor.matmul(
                                  ps[:],
                                  wqkv_sb[:, dc, oc * 128 : (oc + 1) * 128],
                                  xlnT[:, dc, :],
                                  start=(dc == 0),
                                  stop=(dc == ND - 1),
                              )
                          if kind < 2:
                              nc.vector.tensor_scalar_add(
                                  out=qkstage[:, 2 * j + kind, :],
                                  in0=ps[:],
                                  scalar1=bqkv_sb[:, oc : oc + 1],
                              )
                          else:
                              vsb = asm.tile([128, 512], BF16, tag="vsb")
                              nc.vector.tensor_scalar_add(
                                  out=vsb[:], in0=ps[:], scalar1=bqkv_sb[:, oc : oc + 1]
                              )
                              vtp = aps.tile([128, 4, 128], BF16, tag="vtp")
                              for tb in range(4):
                                  nc.tensor.transpose(
                                      vtp[:, tb, :],
                                      vsb[:, tb * 128 : (tb + 1) * 128],
                                      identb[:],
                                  )
                              nc.vector.tensor_copy(vstage[:, j, :, :], vtp[:])
                  for b in range(B):
                      # q/k: one DMA per (batch, kind) covering all 8 dsts
                      for kind in range(2):
                          nc.sync.dma_start(
                              qkv_in[b][:, kind * 128 : (kind + 1) * 128, :]
                              .rearrange("j p c -> p j c"),
                              qkstage[:, :, b * 256 : (b + 1) * 256]
                              .rearrange("p (j k) c -> p j k c", k=2)[:, :, kind, :],
                          )
                      # v (token-major packing): one DMA per batch
                      nc.sync.dma_start(
                          qkv_in[b][:, 256:384, :].rearrange(
                              "j p (t f) -> p j t f", t=2
                          ),
                          vstage[:, :, b * 2 : (b + 1) * 2, :],
                      )
                  if MERGE_A2A:
                      if FAKE_A2A:
                          nc.sync.dma_start(qkv_out_m[:], qkv_in_m[:])
                      else:
                          nc.gpsimd.collective_compute(
                              "AllToAll",
                              OP.bypass,
                              replica_groups=[list(range(N_CORES))],
                              ins=[qkv_in_m[:]],
                              outs=[qkv_out_m[:]],
                          )
                  else:
                      for b in range(B):
                          if FAKE_A2A:
                              nc.sync.dma_start(qkv_out[b][:], qkv_in[b][:])
                          else:
                              nc.gpsimd.collective_compute(
                                  "AllToAll",
                                  OP.bypass,
                                  replica_groups=[list(range(N_CORES))],
                                  ins=[qkv_in[b][:]],
                                  outs=[qkv_out[b][:]],
                              )

              # ---------- phase B: attention (my 2 heads, full sequence) ----------
              with (
                  tc.tile_pool(name=f"b_qk{rep}", bufs=2, side="right") as bqk,
                  tc.tile_pool(name=f"b_sm{rep}", bufs=4, side="right") as bsm,
                  tc.tile_pool(name=f"b_ps{rep}", bufs=2, space="PSUM") as bps,
              ):
                  for b in range(B):
                      qT = bqk.tile([128, S], BF16, tag="qT", name=f"qT{b}")
                      kT = bqk.tile([128, S], BF16, tag="kT", name=f"kT{b}")
                      nc.sync.dma_start(
                          qT[:].rearrange("p (i c) -> p i c", i=N_CORES),
                          qkv_out[b][:, 0:128, :].rearrange("i p c -> p i c"),
                      )
                      nc.sync.dma_start(
                          kT[:].rearrange("p (i c) -> p i c", i=N_CORES),
                          qkv_out[b][:, 128:256, :].rearrange("i p c -> p i c"),
                      )
                      vload = bqk.tile([128, N_CORES, 256], BF16, tag="vload")
                      nc.sync.dma_start(
                          vload[:],
                          qkv_out[b][:, 256:384, :].rearrange("i p c -> p i c"),
                      )
                      for h in range(HPC):
                          nc.vector.tensor_copy(
                              vt[b][h][:, :, 0:DH].rearrange(
                                  "p (i t) f -> p i t f", t=2
                              ),
                              vload[:].rearrange("p i (t f) -> p i t f", t=2)[
                                  :, :, :, h * DH : (h + 1) * DH
                              ],
                          )
                      znall = bqk.tile([128, NQC, 512], BF16, tag="znall", name=f"zn{b}")
                      for qc in range(NQC):
                          nkc = 4 * qc + 4
                          hss = [slice(h * DH, (h + 1) * DH) for h in range(HPC)]
                          zps = [
                              bps.tile([DH + 1, 512], F32, tag="zpsum", name=f"zp{h}")
                              for h in range(HPC)
                          ]
                          for kc in range(nkc):
                              sp2 = bps.tile([128, 1024], F32, tag="sp2", bufs=3)
                              for h in range(HPC):
                                  nc.tensor.matmul(
                                      sp2[:, h * 512 : (h + 1) * 512],
                                      kT[hss[h], kc * 128 : (kc + 1) * 128],
                                      qT[hss[h], qc * 512 : (qc + 1) * 512],
                                      start=True,
                                      stop=True,
                                  )
                              es2 = bsm.tile([128, 1024], BF16, tag="es2", bufs=6)
                              p = kc - 4 * qc  # diagonal-band offset if >= 0
                              if p > 0:
                                  # columns [0:128p] of each half are fully
                                  # causally masked: zero them, skip the exp
                                  nc.vector.memset(
                                      es2[:].rearrange("x (h c) -> x h c", h=2)[
                                          :, :, 0 : 128 * p
                                      ],
                                      0.0,
                                  )
                                  nc.scalar.activation(
                                      es2[:].rearrange("x (h c) -> x h c", h=2)[
                                          :, :, 128 * p : 512
                                      ],
                                      sp2[:].rearrange("x (h c) -> x h c", h=2)[
                                          :, :, 128 * p : 512
                                      ],
                                      AF.Exp,
                                  )
                              else:
                                  nc.scalar.activation(es2[:], sp2[:], AF.Exp)
                              if p >= 0:
                                  # triangular band: cols [128p : 128p+128]
                                  band = es2[:].rearrange("x (h c) -> x h c", h=2)[
                                      :, :, 128 * p : 128 * p + 128
                                  ]
                                  nc.vector.tensor_tensor(
                                      band,
                                      band,
                                      mask_sb[:].rearrange("x (h c) -> x h c", h=2),
                                      OP.mult,
                                  )
                              for h in range(HPC):
                                  nc.tensor.matmul(
                                      zps[h][:],
                                      vt[b][h][:, kc, :],
                                      es2[:, h * 512 : (h + 1) * 512],
                                      start=(kc == 0),
                                      stop=(kc == nkc - 1),
                                  )
                          bct = bps.tile([128, 1024], F32, tag="sp2", bufs=3, name="bct")
                          for h in range(HPC):
                              recip = bsm.tile([1, 512], BF16, tag="recip")
                              with nc.allow_low_precision(
                                  reason="bf16 softmax denom; ~0.4% on z, tiny abs"
                              ):
                                  nc.vector.reciprocal(recip[:], zps[h][DH : DH + 1, :])
                              nc.tensor.matmul(
                                  bct[h * DH : (h + 1) * DH, 0:512],
                                  ones_b[:, 0:DH],
                                  recip[:],
                                  start=True,
                                  stop=True,
                              )
                              bcr = bsm.tile([DH, 512], F32, tag="bcr")
                              nc.vector.tensor_copy(bcr[:], bct[h * DH : (h + 1) * DH, 0:512])
                              with nc.allow_low_precision(
                                  reason="bf16 z for the A2A wire; feeds only o-proj"
                              ):
                                  nc.vector.tensor_tensor(
                                      znall[h * DH : (h + 1) * DH, qc, :],
                                      zps[h][0:DH, :],
                                      bcr[:],
                                      OP.mult,
                                  )
                      nc.sync.dma_start(
                          z_in[b][:].rearrange("(q s) hp c -> hp q s c", s=2),
                          znall[:].rearrange("hp q (s c) -> hp q s c", s=2),
                      )
                      if MERGE_A2A:
                          if b == B - 1:
                              if FAKE_A2A:
                                  nc.sync.dma_start(z_out_m[:], z_in_m[:])
                              else:
                                  nc.gpsimd.collective_compute(
                                      "AllToAll",
                                      OP.bypass,
                                      replica_groups=[list(range(N_CORES))],
                                      ins=[z_in_m[:]],
                                      outs=[z_out_m[:]],
                                  )
                      elif FAKE_A2A:
                          nc.sync.dma_start(z_out[b][:], z_in[b][:])
                      else:
                          nc.gpsimd.collective_compute(
                              "AllToAll",
                              OP.bypass,
                              replica_groups=[list(range(N_CORES))],
                              ins=[z_in[b][:]],
                              outs=[z_out[b][:]],
                          )

              # ---------------- phase C: post (token-parallel) ----------------
              with (
                  tc.tile_pool(name=f"post_w{rep}", bufs=4 if MLP_FP8 else 2, side="right") as pw,
                  tc.tile_pool(name=f"post_big{rep}", bufs=1, side="right") as pbig,
                  tc.tile_pool(name=f"post_t{rep}", bufs=3, side="right") as pt,
                  tc.tile_pool(name=f"post_ps{rep}", bufs=2, space="PSUM") as pps,
              ):
                  resid2 = pbig.tile([128, 4, D], F32, tag="resid2")

                  # broadcast b_out across partitions once: [1, D] -> [128, D]
                  bout_full = pbig.tile([128, D], F32, tag="bout_full")
                  for dc2 in range(2):
                      bps_ = pps.tile([128, 512], F32, tag="ps1")
                      nc.tensor.matmul(
                          bps_[:],
                          ones_sb[:],
                          bout_sb[:, dc2 * 512 : (dc2 + 1) * 512],
                          start=True,
                          stop=True,
                      )
                      nc.vector.tensor_copy(bout_full[:, dc2 * 512 : (dc2 + 1) * 512], bps_[:])

                  # o-projection + residual add -> resid2
                  with tc.tile_pool(name=f"post_o{rep}", bufs=1) as po:
                      zt = po.tile([128, N_CORES, 512], BF16, tag="zt")
                      for b in range(B):
                          nc.scalar.dma_start(
                              zt[:, :, b * 256 : (b + 1) * 256],
                              z_out[b][:].rearrange("i p c -> p i c"),
                          )
                      rsall = pbig.tile([128, 4, D], F32, tag="rsall")
                      nc.scalar.dma_start(
                          rsall[:], resid_mine.rearrange("(t p) d -> p t d", p=128)
                      )
                      for tsub in range(4):
                          rs = rsall[:, tsub, :]
                          for dc2 in range(2):
                              op_ = pps.tile([128, 512], F32, tag="ps1")
                              for hd in range(ND):
                                  nc.tensor.matmul(
                                      op_[:],
                                      zt[:, hd, tsub * 128 : (tsub + 1) * 128],
                                      wo_sb[:, hd, dc2 * 512 : (dc2 + 1) * 512],
                                      start=(hd == 0),
                                      stop=(hd == ND - 1),
                                  )
                              nc.vector.tensor_tensor(
                                  resid2[:, tsub, dc2 * 512 : (dc2 + 1) * 512],
                                  op_[:],
                                  rsall[:, tsub, dc2 * 512 : (dc2 + 1) * 512],
                                  OP.add,
                              )

                  # LN2 + transpose -> xln2T [128, ND, 512] (one batched Sqrt)
                  MDT = FP8 if MLP_FP8 else BF16
                  xln2T = pbig.tile([128, ND, 512], MDT, tag="xln2T")
                  mvs2 = pbig.tile([128, 4, 2], F32, tag="mvs2")
                  for tsub in range(4):
                      stats = pt.tile([128, 2, 6], F32, tag="stats2")
                      nc.vector.bn_stats(stats[:, 0, :], resid2[:, tsub, 0:512])
                      nc.vector.bn_stats(stats[:, 1, :], resid2[:, tsub, 512:1024])
                      nc.vector.bn_aggr(mvs2[:, tsub, :], stats[:])
                  stds2 = pbig.tile([128, 4], F32, tag="stds2")
                  nc.scalar.activation(stds2[:], mvs2[:, :, 1], AF.Sqrt, bias=eps_sb[:])
                  rstds2 = pbig.tile([128, 4], F32, tag="rstds2")
                  nc.vector.reciprocal(rstds2[:], stds2[:])
                  for tsub in range(4):
                      xln2 = pt.tile([128, D], BF16, tag="xln2")
                      nc.vector.tensor_scalar(
                          out=xln2[:],
                          in0=resid2[:, tsub, :],
                          scalar1=mvs2[:, tsub, 0:1],
                          scalar2=rstds2[:, tsub : tsub + 1],
                          op0=OP.subtract,
                          op1=OP.mult,
                      )
                      tpb = pps.tile([128, ND, 128], BF16, tag="ps1t")
                      for dc in range(ND):
                          nc.tensor.transpose(
                              tpb[:, dc, :], xln2[:, dc * 128 : (dc + 1) * 128], identb[:]
                          )
                      nc.vector.tensor_copy(
                          xln2T[:, :, tsub * 128 : (tsub + 1) * 128], tpb[:]
                      )

                  # MLP pass A: h1^T per m-chunk -> gelu -> gT; accumulate out d 0:512
                  gT = pbig.tile([128, NM, 512], MDT, tag="gT")
                  acc = pps.tile([128, 4, 512], F32, tag="acc", bufs=1)
                  for mq in range(NM // 4):
                      wi = pw.tile([128, ND, 512], MDT, tag="wi")
                      nc.sync.dma_start(
                          wi[:], win[mq].rearrange("c p f -> p c f")
                      )
                      wu = pw.tile([128, 4, 512], MDT, tag="wu")
                      nc.sync.dma_start(
                          wu[:],
                          wout[4 * mq : 4 * mq + 4, :, 0:512].rearrange(
                              "m p f -> p m f"
                          ),
                      )
                      for mi in range(4):
                          m = 4 * mq + mi
                          h1 = pps.tile([128, 512], F32, tag="ps1")
                          if MLP_FP8:
                              for dcp in range(ND // 2):
                                  nc.tensor.matmul(
                                      h1[:],
                                      wi[:, 2 * dcp : 2 * dcp + 2, mi * 128 : (mi + 1) * 128],
                                      xln2T[:, 2 * dcp : 2 * dcp + 2, :],
                                      start=(dcp == 0),
                                      stop=(dcp == ND // 2 - 1),
                                      perf_mode=mybir.MatmulPerfMode.DoubleRow,
                                  )
                          else:
                              for dc in range(ND):
                                  nc.tensor.matmul(
                                      h1[:],
                                      wi[:, dc, mi * 128 : (mi + 1) * 128],
                                      xln2T[:, dc, :],
                                      start=(dc == 0),
                                      stop=(dc == ND - 1),
                                  )
                          with nc.allow_low_precision(
                              reason="fp8 MLP activations; ~8.5e-3 rel err measured"
                          ):
                              nc.scalar.activation(
                                  gT[:, m, :], h1[:], AF.Gelu_apprx_tanh,
                                  bias=bin_sb[:, m : m + 1],
                                  scale=1.0 / WS_MLP,
                              )
                      if MLP_FP8:
                          for mi2 in range(2):
                              for tsub in range(4):
                                  nc.tensor.matmul(
                                      acc[:, tsub, :],
                                      gT[
                                          :,
                                          4 * mq + 2 * mi2 : 4 * mq + 2 * mi2 + 2,
                                          tsub * 128 : (tsub + 1) * 128,
                                      ],
                                      wu[:, 2 * mi2 : 2 * mi2 + 2, :],
                                      start=(mq == 0 and mi2 == 0),
                                      stop=(mq == NM // 4 - 1 and mi2 == 1),
                                      perf_mode=mybir.MatmulPerfMode.DoubleRow,
                                  )
                      else:
                          for mi in range(4):
                              for tsub in range(4):
                                  nc.tensor.matmul(
                                      acc[:, tsub, :],
                                      gT[
                                          :,
                                          4 * mq + mi,
                                          tsub * 128 : (tsub + 1) * 128,
                                      ],
                                      wu[:, mi, :],
                                      start=(mq == 0 and mi == 0),
                                      stop=(mq == NM // 4 - 1 and mi == 3),
                                  )
                  otall = pbig.tile([128, 4, 512], F32, tag="otall")
                  for tsub in range(4):
                      nc.vector.scalar_tensor_tensor(
                          otall[:, tsub, :], acc[:, tsub, :], 1.0 / WS_MLP,
                          resid2[:, tsub, 0:512], OP.mult, OP.add,
                      )
                      nc.vector.tensor_tensor(
                          otall[:, tsub, :], otall[:, tsub, :], bout_full[:, 0:512], OP.add
                      )
                  nc.scalar.dma_start(
                      y[:, 0:512].rearrange("(t p) f -> p t f", p=128), otall[:]
                  )

                  # MLP pass B: same gT, out d 512:1024
                  acc2 = pps.tile([128, 4, 512], F32, tag="acc", bufs=1)
                  for mq in range(NM // 4):
                      wu = pw.tile([128, 4, 512], MDT, tag="wu")
                      nc.sync.dma_start(
                          wu[:],
                          wout[4 * mq : 4 * mq + 4, :, 512:1024].rearrange(
                              "m p f -> p m f"
                          ),
                      )
                      if MLP_FP8:
                          for mi2 in range(2):
                              for tsub in range(4):
                                  nc.tensor.matmul(
                                      acc2[:, tsub, :],
                                      gT[
                                          :,
                                          4 * mq + 2 * mi2 : 4 * mq + 2 * mi2 + 2,
                                          tsub * 128 : (tsub + 1) * 128,
                                      ],
                                      wu[:, 2 * mi2 : 2 * mi2 + 2, :],
                                      start=(mq == 0 and mi2 == 0),
                                      stop=(mq == NM // 4 - 1 and mi2 == 1),
                                      perf_mode=mybir.MatmulPerfMode.DoubleRow,
                                  )
                      else:
                          for mi in range(4):
                              for tsub in range(4):
                                  nc.tensor.matmul(
                                      acc2[:, tsub, :],
                                      gT[
                                          :,
                                          4 * mq + mi,
                                          tsub * 128 : (tsub + 1) * 128,
                                      ],
                                      wu[:, mi, :],
                                      start=(mq == 0 and mi == 0),
                                      stop=(mq == NM // 4 - 1 and mi == 3),
                                  )
                  otall2 = pbig.tile([128, 4, 512], F32, tag="otall")
                  for tsub in range(4):
                      nc.vector.scalar_tensor_tensor(
                          otall2[:, tsub, :], acc2[:, tsub, :], 1.0 / WS_MLP,
                          resid2[:, tsub, 512:1024], OP.mult, OP.add,
                      )
                      nc.vector.tensor_tensor(
                          otall2[:, tsub, :], otall2[:, tsub, :], bout_full[:, 512:1024], OP.add
                      )
                  nc.scalar.dma_start(
                      y[:, 512:1024].rearrange("(t p) f -> p t f", p=128), otall2[:]
                  )

    nc.compile()
    return nc


def _prep_inputs(inputs):
    """Host-side weight folding; returns per-core in_maps."""
    f32 = np.float32
    resid = np.asarray(inputs["resid"], f32)
    w_q = np.asarray(inputs["w_q"], f32)
    w_k = np.asarray(inputs["w_k"], f32)
    w_v = np.asarray(inputs["w_v"], f32)
    w_o = np.asarray(inputs["w_o"], f32)
    ln1_w = np.asarray(inputs["ln1_w"], f32)
    ln1_b = np.asarray(inputs["ln1_b"], f32)
    ln2_w = np.asarray(inputs["ln2_w"], f32)
    ln2_b = np.asarray(inputs["ln2_b"], f32)
    w_in = np.asarray(inputs["w_in"], f32)
    b_in = np.asarray(inputs["b_in"], f32)
    w_out = np.asarray(inputs["w_out"], f32)
    b_out = np.asarray(inputs["b_out"], f32)

    sm = 1.0 / np.sqrt(DH)
    win_f = ln2_w[:, None] * w_in  # [D, DM]
    bin_f = ln2_b @ w_in + b_in  # [DM]

    # [NM//2, ND, 128, 256]: pair m-chunks (2mp, 2mp+1) side by side in f;
    # scaled by WS_MLP so fp8e4m3 sees a good exponent range (undone in gelu)
    mdt = ml_dtypes.float8_e4m3 if MLP_FP8 else ml_dtypes.bfloat16
    win_host = np.ascontiguousarray(
        (win_f * WS_MLP)
        .reshape(ND, 128, NM // 4, 4, 128)
        .transpose(2, 0, 1, 3, 4)
        .reshape(NM // 4, ND, 128, 512)
        .astype(mdt)
    )
    # bin layout [128, NM]: column m holds, on partition p, the bias of
    # m-feature 128*m + p (gelu bias is per-partition of the h1 psum tile).
    bin_host = np.ascontiguousarray(bin_f.reshape(NM, 128).T)
    wout_host = np.ascontiguousarray(
        (w_out * WS_MLP).reshape(NM, 128, D).astype(mdt)
    )
    wo_host = np.ascontiguousarray(
        w_o.reshape(H * DH, D).reshape(ND, 128, D).astype(ml_dtypes.bfloat16)
    )
    bout_host = np.ascontiguousarray(b_out.reshape(1, D))

    ones_host = np.ones((1, 128), f32)

    # QKV weights, dst-major: for dst core j: q(heads 2j,2j+1) | k | v, 128
    # cols each. LN1 gain and the softmax scale are folded in.
    wq_f = (ln1_w[:, None, None] * w_q.transpose(1, 0, 2) * sm).reshape(D, D)
    wk_f = (ln1_w[:, None, None] * w_k.transpose(1, 0, 2)).reshape(D, D)
    wv_f = (ln1_w[:, None, None] * w_v.transpose(1, 0, 2)).reshape(D, D)
    bq_f = (ln1_b @ w_q.transpose(1, 0, 2).reshape(D, D)) * sm
    bk_f = ln1_b @ w_k.transpose(1, 0, 2).reshape(D, D)
    bv_f = ln1_b @ w_v.transpose(1, 0, 2).reshape(D, D)
    wcols = []
    bcols = []
    for j in range(N_CORES):
        fs = slice(j * 128, (j + 1) * 128)
        wcols += [wq_f[:, fs], wk_f[:, fs], wv_f[:, fs]]
        bcols += [bq_f[fs], bk_f[fs], bv_f[fs]]
    wqkv_host = np.ascontiguousarray(
        np.concatenate(wcols, axis=1).reshape(ND, 128, 3 * D).astype(ml_dtypes.bfloat16)
    )
    bqkv_host = np.ascontiguousarray(np.stack(bcols).reshape(NOC, 128, 1))

    tri = (np.arange(128)[:, None] <= np.arange(128)[None, :]).astype(
        ml_dtypes.bfloat16
    )
    masks_host = np.ascontiguousarray(np.concatenate([tri, tri], axis=1))
    ident_host = np.eye(128, dtype=f32)

    in_maps = []
    for c in range(N_CORES):
        t0 = TPB * c
        rm = np.concatenate(
            [resid[0, t0 : t0 + TPB], resid[1, t0 : t0 + TPB]], axis=0
        )
        in_maps.append(
            {
                "resid_mine": np.ascontiguousarray(rm),
                "wqkv": wqkv_host,
                "bqkv": bqkv_host,
                "wo": wo_host,
                "win": win_host,
                "bin": bin_host,
                "wout": wout_host,
                "bout": bout_host,
                "masks": masks_host,
                "ident": ident_host,
                "ones": ones_host,
            }
        )
    return in_maps


class _Runner:
    """Compile once; keep the jitted shard_map callable and device-resident
    inputs so repeat executes measure the kernel, not host overhead."""

    def __init__(self):
        import jax
        from concourse import bass2jax

        self.jax = jax
        self.bass2jax = bass2jax
        bass2jax.install_neuronx_cc_hook()
        nc = build_nc()
        self.nc = nc

        in_names, out_names, out_avals, zero_shapes = [], [], [], []
        for alloc in nc.m.functions[0].allocations:
            if not isinstance(alloc, mybir.MemoryLocationSet):
                continue
            name = alloc.memorylocations[0].name
            if alloc.kind == "ExternalInput":
                if not (nc.partition_id_tensor and name == nc.partition_id_tensor.name):
                    in_names.append(name)
            elif alloc.kind == "ExternalOutput":
                shape = tuple(alloc.tensor_shape)
                dtype = mybir.dt.np(alloc.dtype)
                out_names.append(name)
                out_avals.append(jax.core.ShapedArray(shape, dtype))
                zero_shapes.append((shape, dtype))
        n_params = len(in_names)
        all_in_names = list(in_names) + list(out_names)
        partition_name = (
            nc.partition_id_tensor.name if nc.partition_id_tensor else None
        )
        if partition_name is not None:
            all_in_names.append(partition_name)
        self.in_names = in_names
        self.out_names = out_names
        self.zero_shapes = zero_shapes
        n_outs = len(out_names)

        def _body(*args):
            operands = list(args)
            if partition_name is not None:
                operands.append(bass2jax.partition_id_tensor())
            outs = bass2jax._bass_exec_p.bind(
                *operands,
                out_avals=tuple(out_avals),
                in_names=tuple(all_in_names),
                out_names=tuple(out_names),
                lowering_input_output_aliases=(),
                sim_require_finite=True,
                sim_require_nnan=True,
                nc=nc,
            )
            return tuple(outs)

        from jax.sharding import Mesh, NamedSharding, PartitionSpec
        from jax.experimental.shard_map import shard_map

        devices = jax.devices()[:N_CORES]
        self.mesh = Mesh(np.asarray(devices), ("core",))
        self.sharding = NamedSharding(self.mesh, PartitionSpec("core"))
        donate = tuple(range(n_params, n_params + n_outs))
        in_specs = (PartitionSpec("core"),) * (n_params + n_outs)
        out_specs = (PartitionSpec("core"),) * n_outs
        self.sharded = jax.jit(
            shard_map(
                _body,
                mesh=self.mesh,
                in_specs=in_specs,
                out_specs=out_specs,
                check_rep=False,
            ),
            donate_argnums=donate,
            keep_unused=True,
        )

    def put_inputs(self, in_maps):
        concat = [
            np.concatenate([np.asarray(m[name]) for m in in_maps], axis=0)
            for name in self.in_names
        ]
        return [self.jax.device_put(a, self.sharding) for a in concat]

    def _zeros(self):
        return [
            np.zeros((N_CORES * s[0], *s[1:]), dt) for (s, dt) in self.zero_shapes
        ]

    def execute(self, dev_in):
        outs = self.sharded(*dev_in, *self._zeros())
        for o in outs:
            o.block_until_ready()
        return outs

    def gather(self, outs):
        per_core = {}
        for i, name in enumerate(self.out_names):
            arr = np.asarray(outs[i])
            per_core[name] = arr.reshape(N_CORES, -1, *arr.shape[1:])
        return per_core


_RUNNER = None


def _get_runner():
    global _RUNNER
    if _RUNNER is None:
        _RUNNER = _Runner()
    return _RUNNER


def kernel(**inputs) -> np.ndarray:
    r = _get_runner()
    in_maps = _prep_inputs(inputs)
    dev_in = r.put_inputs(in_maps)
    outs = r.execute(dev_in)
    ys = r.gather(outs)["y"]  # [8, 512, 1024]
    out = np.zeros((B, S, D), np.float32)
    for c in range(N_CORES):
        out[0, TPB * c : TPB * c + TPB] = ys[c][0:TPB]
        out[1, TPB * c : TPB * c + TPB] = ys[c][TPB : 2 * TPB]
    return out


if __name__ == "__main__":
    # quick self-exercise with random data
    rng = np.random.default_rng(0)
    ins = {
        "resid": rng.standard_normal((B, S, D)).astype(np.float32),
        "w_q": 0.02 * rng.standard_normal((H, D, DH)).astype(np.float32),
        "w_k": 0.02 * rng.standard_normal((H, D, DH)).astype(np.float32),
        "w_v": 0.02 * rng.standard_normal((H, D, DH)).astype(np.float32),
        "w_o": 0.02 * rng.standard_normal((H, DH, D)).astype(np.float32),
        "ln1_w": 0.02 * rng.standard_normal(D).astype(np.float32),
        "ln1_b": np.zeros(D, np.float32),
        "ln2_w": 0.02 * rng.standard_normal(D).astype(np.float32),
        "ln2_b": np.zeros(D, np.float32),
        "w_in": 0.02 * rng.standard_normal((D, DM)).astype(np.float32),
        "b_in": np.zeros(DM, np.float32),
        "w_out": 0.02 * rng.standard_normal((DM, D)).astype(np.float32),
        "b_out": np.zeros(D, np.float32),
    }
    out = kernel(**ins)
    print("out", out.shape, out.dtype, float(np.abs(out).mean()))
